# revision 41
# baseline (speedup 1.0000x reference)
"""Trainium2 Bass kernel for nn_BasisDense: y = einsum('bd,duk,bk->bu', x, kernel, c_prob) + bias.

Strategy (v5):
  - Factorize: t[b,(u,k)] = x @ kernel2d (kernel2d = kernel.reshape(D, U*K)),
    then y[b,u] = sum_k t[b,u,k]*c_prob[b,k] (DVE epilogue); bias added on host.
  - All-bf16 matmuls (full PE rate, 216ns/512-row measured issue cadence; fp8
    DoubleRow measured at parity -- the moving stream is 1 elem/cycle either
    way -- so it is disabled by default via M_LIST). Operands pre-scaled by
    powers of two (x*16, kernel*256 -- exact in bf16) with the 1/4096 unscale
    folded into c_prob on the host, so an fp8 head can share the PSUM group.
  - Hybrid shard across 8 cores: batch B into 4 x units U into 2.
  - Queue discipline: each DMA-trigger ring costs ~600ns of engine time, so
    the kernel chunk stream is split across BOTH HWDGE queues (sync/scalar,
    alternating chunks) to keep ring pace ~300ns/chunk; x rides along in
    consumption order; y rings (which wait on the epilogue semaphore) go to
    the otherwise-idle gpsimd SWDGE queue with a deep yt pool to absorb its
    latency, so no input stream ever sits behind a semaphore wait.
  - ~13 warm-up matmuls on a gpsimd-memset tile run during the DMA head so
    the PE_HAM clock gate is released (2.4 GHz) before the real stream.
"""
import sys

sys.path.insert(0, "/opt/trn_rl_repo")

import numpy as np
import concourse.bacc as bacc
import concourse.mybir as mybir
import concourse.tile as tile
from concourse import bass_utils

B, D, U, K = 4096, 2048, 2048, 8
NCORES = 8
SHARD_U = 2  # units-dimension shards
SHARD_B = NCORES // SHARD_U
BS = B // SHARD_B  # batch rows per core
USH = U // SHARD_U  # units per core
UKS = USH * K  # fused (u,k) output columns per core
NFREE = 512  # matmul moving free dim (1 PSUM bank of fp32)
NT = UKS // NFREE  # n-tiles
DT = D // 128  # contraction d-tiles
BT = BS // 128  # batch partition-tiles per core
UPT = NFREE // K  # u-columns produced per n-tile
KT_BUFS = 5
WARM_MMS = 8  # PE warm-up matmuls issued during the DMA head

# fp8 d-tiles per n-tile (0, 2 or 4; DoubleRow pairs). A DoubleRow matmul
# measures ~110ns effective vs 432ns for the 2 bf16 matmuls it replaces
# (full 2x rate). rel err: all-2 -> 1.34e-2, [4,2] mix -> 1.63e-2,
# all-4 -> 1.88e-2 (gate 2e-2).
M_LIST = [2, 4] * (NT // 2)
MF8 = 4 if max(M_LIST) > 0 else 0  # d-tiles shipped in fp8
D8 = MF8 * 128  # fp8 d-rows
T16OFF = 2 if MF8 else 0  # first d-tile carried in bf16
DT16 = DT - T16OFF

XSCALE = 16.0  # x pre-scale (power of 2: exact in bf16)
WSCALE = 256.0  # kernel pre-scale
F8 = mybir.dt.float8e4
BF16 = mybir.dt.bfloat16

_CACHE = {}


def _build():
    nc = bacc.Bacc("TRN2", target_bir_lowering=False, debug=False, num_devices=NCORES)
    f32 = mybir.dt.float32
    DR = mybir.MatmulPerfMode.DoubleRow

    xt16 = nc.dram_tensor("xt16", [DT16 * 128, BS], BF16, kind="ExternalInput").ap()
    cp = nc.dram_tensor("cp", [BS, K], f32, kind="ExternalInput").ap()
    kern16 = nc.dram_tensor("kern16", [DT16 * 128, USH, K], BF16, kind="ExternalInput").ap()
    y = nc.dram_tensor("y", [BS, USH], f32, kind="ExternalOutput").ap()
    if MF8:
        xt8 = nc.dram_tensor("xt8", [D8, BS], F8, kind="ExternalInput").ap()
        kern8 = nc.dram_tensor("kern8", [D8, USH, K], F8, kind="ExternalInput").ap()
        x8v = xt8.rearrange("(t p) b -> p t b", p=128)  # [128, MF8, BS]
        k8v = kern8.rearrange("(t p) u k -> p t (u k)", p=128)  # [128, MF8, UKS]

    x16v = xt16.rearrange("(t p) b -> p t b", p=128)  # [128, DT16, BS]
    k16v = kern16.rearrange("(t p) u k -> p t (u k)", p=128)  # [128, DT16, UKS]

    with tile.TileContext(nc) as tc:
        with (
            tc.tile_pool(name="const", bufs=1) as constp,
            tc.tile_pool(name="kt8p", bufs=KT_BUFS) as ktp8,
            tc.tile_pool(name="kt16p", bufs=KT_BUFS) as ktp16,
            tc.tile_pool(name="mps", bufs=8, space="PSUM") as mps,
            tc.tile_pool(name="ep", bufs=6) as epp,
            tc.tile_pool(name="yp", bufs=32) as ypp,
        ):
            if MF8:
                xT8 = constp.tile([128, MF8, BS], F8)
            xT16 = constp.tile([128, DT16, BS], BF16)
            c_nat = constp.tile([128, BT, K], f32)
            c_rep = constp.tile([128, BT, NFREE], f32)
            warm = constp.tile([128, NFREE], BF16)

            # c_prob first on the sync queue (tiny, needed by bt0 epilogue)
            nc.sync.dma_start(c_nat, cp.rearrange("(bt p) k -> p bt k", p=128))

            # PE warm-up: releases the HAM clock gate during the DMA head.
            # The producer memset rides GpSimd (idle at startup). The warm-up
            # PSUM bank comes from the mps pool and is recycled by the real
            # accumulation groups.
            nc.gpsimd.memset(warm, 0.0)
            wpsum = mps.tile([128, NFREE], f32, tag="acc")
            for _ in range(WARM_MMS):
                nc.tensor.matmul(
                    wpsum, warm[:, 0:128], warm, start=True, stop=True,
                    skip_group_check=True,
                )

            # replicate c_prob 64x along the free dim on the DVE (tiny)
            for bt in range(BT):
                nc.vector.tensor_copy(c_rep[:, bt, 0:K], c_nat[:, bt, :])
                s = K
                while s < NFREE:
                    nc.vector.tensor_copy(c_rep[:, bt, s : 2 * s], c_rep[:, bt, 0:s])
                    s *= 2

            def fetch(n, first=False):
                m = M_LIST[n]
                kt8 = None
                if MF8:
                    kt8 = ktp8.tile([128, MF8, NFREE], F8, tag="kt8")
                    if m:
                        nc.scalar.dma_start(
                            kt8[:, 0:m, :],
                            k8v[:, 0:m, n * NFREE : (n + 1) * NFREE],
                        )
                kt16 = ktp16.tile([128, DT16, NFREE], BF16, tag="kt16")
                nsl = slice(n * NFREE, (n + 1) * NFREE)
                if first:
                    # n0 is fine-grained and just-in-time: the first DoubleRow
                    # matmul is gated on xt8/kt8, so ring those before the
                    # per-tile ride-along stream
                    if MF8:
                        nc.sync.dma_start(xT8[:, :, 0:128], x8v[:, :, 0:128])
                    for t in range(m, DT):
                        eng = nc.sync if t % 2 == 0 else nc.scalar
                        oth = nc.scalar if t % 2 == 0 else nc.sync
                        eng.dma_start(kt16[:, t - m, :], k16v[:, t - T16OFF, nsl])
                        # bt0's x slice chunks pace along on the other queue
                        oth.dma_start(
                            xT16[:, t - T16OFF, 0:128], x16v[:, t - T16OFF, 0:128]
                        )
                    # x chunks not covered by the ride-along loop above
                    # (d-tiles below m that only smaller-m n-tiles read)
                    for idx in range(0, m - T16OFF):
                        eng = nc.sync if idx % 2 == 0 else nc.scalar
                        eng.dma_start(xT16[:, idx, 0:128], x16v[:, idx, 0:128])
                else:
                    # steady state: coarse 4-tile chunks, ~600ns of engine
                    # time per ring makes fewer+bigger rings strictly better
                    # (prefetch depth covers the arrival granularity)
                    ntiles = DT - m
                    q = 0
                    for lo in range(0, ntiles, 4):
                        hi = min(lo + 4, ntiles)
                        eng = nc.sync if q % 2 == 0 else nc.scalar
                        q += 1
                        eng.dma_start(
                            kt16[:, lo:hi, :], k16v[:, m - T16OFF + lo : m - T16OFF + hi, nsl]
                        )
                return kt8, kt16

            kt_first = fetch(0, first=True)

            # remaining x slices (bt 1..7), alternating queues
            for bt in range(1, BT):
                bsl = slice(bt * 128, (bt + 1) * 128)
                eng = nc.sync if bt % 2 == 0 else nc.scalar
                eng.dma_start(xT16[:, :, bsl], x16v[:, :, bsl])
                if MF8:
                    (nc.scalar if bt % 2 == 0 else nc.sync).dma_start(
                        xT8[:, :, bsl], x8v[:, :, bsl]
                    )

            for n in range(NT):
                m = M_LIST[n]
                kt8, kt16 = kt_first if n == 0 else fetch(n)
                for bt in range(BT):
                    bsl = slice(bt * 128, (bt + 1) * 128)
                    acc = mps.tile([128, NFREE], f32, tag="acc")
                    for j in range(m // 2):
                        nc.tensor.matmul(
                            acc,
                            xT8[:, 2 * j : 2 * j + 2, bsl],
                            kt8[:, 2 * j : 2 * j + 2, :],
                            start=(j == 0),
                            stop=False,
                            perf_mode=DR,
                        )
                    for t in range(m, DT):
                        nc.tensor.matmul(
                            acc,
                            xT16[:, t - T16OFF, bsl],
                            kt16[:, t - m, :],
                            start=(m == 0 and t == 0),
                            stop=(t == DT - 1),
                        )
                    # epilogue: y[b, u] = sum_k acc[b, (u,k)] * c[b, k]
                    tmp = epp.tile([128, NFREE], f32, tag="tmp")
                    nc.vector.tensor_mul(tmp, acc, c_rep[:, bt, :])
                    yt = ypp.tile([128, UPT], f32, tag="yt")
                    nc.vector.tensor_reduce(
                        yt,
                        tmp.rearrange("p (u k) -> p u k", k=K),
                        axis=mybir.AxisListType.X,
                        op=mybir.AluOpType.add,
                    )
                    # y rings ride the gpsimd SWDGE queue: their wait on the
                    # epilogue semaphore must not gate the input streams, and
                    # the deep yt pool absorbs SWDGE latency
                    nc.gpsimd.dma_start(
                        y[bsl, n * UPT : (n + 1) * UPT],
                        yt,
                    )
    nc.compile()
    return nc


def _in_maps(x, c_prob, kernel, bias):
    np16 = mybir.dt.np(BF16)
    x = np.asarray(x, dtype=np.float32)
    c_prob = np.asarray(c_prob, dtype=np.float32)
    kernel = np.asarray(kernel, dtype=np.float32)

    xs = (x.T * np.float32(XSCALE))  # [D, B]
    x16_full = xs[T16OFF * 128 :].astype(np16)
    ks = kernel * np.float32(WSCALE)
    k16_full = ks[T16OFF * 128 :].astype(np16)
    cps = c_prob * np.float32(1.0 / (XSCALE * WSCALE))
    if MF8:
        np8 = mybir.dt.np(F8)
        x8_full = xs[:D8].astype(np8)
        k8_full = ks[:D8].astype(np8)  # [D8, U, K]

    maps = []
    for c in range(NCORES):
        bq, uh = c % SHARD_B, c // SHARD_B
        bsl = slice(bq * BS, (bq + 1) * BS)
        usl = slice(uh * USH, (uh + 1) * USH)
        mp = {
            "xt16": np.ascontiguousarray(x16_full[:, bsl]),
            "cp": np.ascontiguousarray(cps[bsl]),
            "kern16": np.ascontiguousarray(k16_full[:, usl, :]),
        }
        if MF8:
            mp["xt8"] = np.ascontiguousarray(x8_full[:, bsl])
            mp["kern8"] = np.ascontiguousarray(k8_full[:, usl, :])
        maps.append(mp)
    return maps


def _gather(results, bias):
    out = np.empty((B, U), dtype=np.float32)
    for c in range(NCORES):
        bq, uh = c % SHARD_B, c // SHARD_B
        out[bq * BS : (bq + 1) * BS, uh * USH : (uh + 1) * USH] = results[c]["y"]
    out += np.asarray(bias, dtype=np.float32)
    return out


def kernel(x, c_prob, kernel, bias):
    if "nc" not in _CACHE:
        _CACHE["nc"] = _build()
    nc = _CACHE["nc"]
    res = bass_utils.run_bass_kernel_spmd(
        nc, _in_maps(x, c_prob, kernel, bias), list(range(NCORES))
    )
    return _gather(res.results, bias)


# revision 43
# speedup vs baseline: 1.0062x; 1.0062x over previous
"""Trainium2 Bass kernel for nn_BasisDense: y = einsum('bd,duk,bk->bu', x, kernel, c_prob) + bias.

Strategy (v5):
  - Factorize: t[b,(u,k)] = x @ kernel2d (kernel2d = kernel.reshape(D, U*K)),
    then y[b,u] = sum_k t[b,u,k]*c_prob[b,k] (DVE epilogue); bias added on host.
  - All-bf16 matmuls (full PE rate, 216ns/512-row measured issue cadence; fp8
    DoubleRow measured at parity -- the moving stream is 1 elem/cycle either
    way -- so it is disabled by default via M_LIST). Operands pre-scaled by
    powers of two (x*16, kernel*256 -- exact in bf16) with the 1/4096 unscale
    folded into c_prob on the host, so an fp8 head can share the PSUM group.
  - Hybrid shard across 8 cores: batch B into 4 x units U into 2.
  - Queue discipline: each DMA-trigger ring costs ~600ns of engine time, so
    the kernel chunk stream is split across BOTH HWDGE queues (sync/scalar,
    alternating chunks) to keep ring pace ~300ns/chunk; x rides along in
    consumption order; y rings (which wait on the epilogue semaphore) go to
    the otherwise-idle gpsimd SWDGE queue with a deep yt pool to absorb its
    latency, so no input stream ever sits behind a semaphore wait.
  - ~13 warm-up matmuls on a gpsimd-memset tile run during the DMA head so
    the PE_HAM clock gate is released (2.4 GHz) before the real stream.
"""
import sys

sys.path.insert(0, "/opt/trn_rl_repo")

import numpy as np
import concourse.bacc as bacc
import concourse.mybir as mybir
import concourse.tile as tile
from concourse import bass_utils

B, D, U, K = 4096, 2048, 2048, 8
NCORES = 8
SHARD_U = 2  # units-dimension shards
SHARD_B = NCORES // SHARD_U
BS = B // SHARD_B  # batch rows per core
USH = U // SHARD_U  # units per core
UKS = USH * K  # fused (u,k) output columns per core
NFREE = 512  # matmul moving free dim (1 PSUM bank of fp32)
NT = UKS // NFREE  # n-tiles
DT = D // 128  # contraction d-tiles
BT = BS // 128  # batch partition-tiles per core
UPT = NFREE // K  # u-columns produced per n-tile
KT_BUFS = 5
WARM_MMS = 8  # PE warm-up matmuls issued during the DMA head

# fp8 d-tiles per n-tile (0, 2 or 4; DoubleRow pairs). A DoubleRow matmul
# measures ~110ns effective vs 432ns for the 2 bf16 matmuls it replaces
# (full 2x rate). rel err: all-2 -> 1.34e-2, [4,2] mix -> 1.63e-2,
# all-4 -> 1.88e-2 (gate 2e-2).
M_LIST = [2, 4] * (NT // 2)
MF8 = 4 if max(M_LIST) > 0 else 0  # d-tiles shipped in fp8
D8 = MF8 * 128  # fp8 d-rows
T16OFF = 2 if MF8 else 0  # first d-tile carried in bf16
DT16 = DT - T16OFF

XSCALE = 16.0  # x pre-scale (power of 2: exact in bf16)
WSCALE = 256.0  # kernel pre-scale
F8 = mybir.dt.float8e4
BF16 = mybir.dt.bfloat16

_CACHE = {}


def _build():
    nc = bacc.Bacc("TRN2", target_bir_lowering=False, debug=False, num_devices=NCORES)
    f32 = mybir.dt.float32
    DR = mybir.MatmulPerfMode.DoubleRow

    xt16 = nc.dram_tensor("xt16", [DT16 * 128, BS], BF16, kind="ExternalInput").ap()
    cp = nc.dram_tensor("cp", [BS, K], f32, kind="ExternalInput").ap()
    kern16 = nc.dram_tensor("kern16", [DT16 * 128, USH, K], BF16, kind="ExternalInput").ap()
    y = nc.dram_tensor("y", [BS, USH], f32, kind="ExternalOutput").ap()
    if MF8:
        xt8 = nc.dram_tensor("xt8", [D8, BS], F8, kind="ExternalInput").ap()
        kern8 = nc.dram_tensor("kern8", [D8, USH, K], F8, kind="ExternalInput").ap()
        x8v = xt8.rearrange("(t p) b -> p t b", p=128)  # [128, MF8, BS]
        k8v = kern8.rearrange("(t p) u k -> p t (u k)", p=128)  # [128, MF8, UKS]

    x16v = xt16.rearrange("(t p) b -> p t b", p=128)  # [128, DT16, BS]
    k16v = kern16.rearrange("(t p) u k -> p t (u k)", p=128)  # [128, DT16, UKS]

    with tile.TileContext(nc) as tc:
        with (
            tc.tile_pool(name="const", bufs=1) as constp,
            tc.tile_pool(name="kt8p", bufs=KT_BUFS) as ktp8,
            tc.tile_pool(name="kt16p", bufs=KT_BUFS) as ktp16,
            tc.tile_pool(name="mps", bufs=8, space="PSUM") as mps,
            tc.tile_pool(name="ep", bufs=6) as epp,
            tc.tile_pool(name="yp", bufs=32) as ypp,
        ):
            if MF8:
                xT8 = constp.tile([128, MF8, BS], F8)
            xT16 = constp.tile([128, DT16, BS], BF16)
            c_nat = constp.tile([128, BT, K], f32)
            c_rep = constp.tile([128, BT, NFREE], f32)
            warm = constp.tile([128, NFREE], BF16)

            # c_prob first on the sync queue (tiny, needed by bt0 epilogue)
            nc.sync.dma_start(c_nat, cp.rearrange("(bt p) k -> p bt k", p=128))

            # PE warm-up: releases the HAM clock gate during the DMA head.
            # The producer memset rides GpSimd (idle at startup). The warm-up
            # PSUM bank comes from the mps pool and is recycled by the real
            # accumulation groups.
            nc.gpsimd.memset(warm, 0.0)
            wpsum = mps.tile([128, NFREE], f32, tag="acc")
            for _ in range(WARM_MMS):
                nc.tensor.matmul(
                    wpsum, warm[:, 0:128], warm, start=True, stop=True,
                    skip_group_check=True,
                )

            # replicate c_prob 64x along the free dim on the DVE (tiny)
            for bt in range(BT):
                nc.vector.tensor_copy(c_rep[:, bt, 0:K], c_nat[:, bt, :])
                s = K
                while s < NFREE:
                    nc.vector.tensor_copy(c_rep[:, bt, s : 2 * s], c_rep[:, bt, 0:s])
                    s *= 2

            def fetch(n, first=False):
                m = M_LIST[n]
                kt8 = None
                if MF8:
                    kt8 = ktp8.tile([128, MF8, NFREE], F8, tag="kt8")
                    if m:
                        nc.scalar.dma_start(
                            kt8[:, 0:m, :],
                            k8v[:, 0:m, n * NFREE : (n + 1) * NFREE],
                        )
                kt16 = ktp16.tile([128, DT16, NFREE], BF16, tag="kt16")
                nsl = slice(n * NFREE, (n + 1) * NFREE)
                if first:
                    # Head schedule, hand-ordered for just-in-time supply of
                    # groups 0-1 while the clock is still cold. The first
                    # DoubleRow matmul is gated on xt8-bt0/kt8 (rung first);
                    # bt0's bf16 x arrives as two half-slices racing the
                    # per-tile kt chunk stream; bt1's slices ride ahead of
                    # the kt chunks so group 1 never waits.
                    half = DT16 // 2
                    if MF8:
                        nc.sync.dma_start(xT8[:, :, 0:128], x8v[:, :, 0:128])
                    nc.scalar.dma_start(xT16[:, 0:half, 0:128], x16v[:, 0:half, 0:128])
                    nc.sync.dma_start(
                        xT16[:, half:DT16, 0:128], x16v[:, half:DT16, 0:128]
                    )
                    nc.scalar.dma_start(xT16[:, :, 128:256], x16v[:, :, 128:256])
                    if MF8:
                        nc.sync.dma_start(xT8[:, :, 128:256], x8v[:, :, 128:256])
                    for t in range(m, DT):
                        eng = nc.sync if t % 2 == 0 else nc.scalar
                        eng.dma_start(kt16[:, t - m, :], k16v[:, t - T16OFF, nsl])
                else:
                    # steady state: coarse 4-tile chunks, ~600ns of engine
                    # time per ring makes fewer+bigger rings strictly better
                    # (prefetch depth covers the arrival granularity)
                    ntiles = DT - m
                    q = 0
                    for lo in range(0, ntiles, 4):
                        hi = min(lo + 4, ntiles)
                        eng = nc.sync if q % 2 == 0 else nc.scalar
                        q += 1
                        eng.dma_start(
                            kt16[:, lo:hi, :], k16v[:, m - T16OFF + lo : m - T16OFF + hi, nsl]
                        )
                return kt8, kt16

            kt_first = fetch(0, first=True)

            # remaining x slices (bt 2..7), alternating queues
            for bt in range(2, BT):
                bsl = slice(bt * 128, (bt + 1) * 128)
                eng = nc.sync if bt % 2 == 0 else nc.scalar
                eng.dma_start(xT16[:, :, bsl], x16v[:, :, bsl])
                if MF8:
                    (nc.scalar if bt % 2 == 0 else nc.sync).dma_start(
                        xT8[:, :, bsl], x8v[:, :, bsl]
                    )

            for n in range(NT):
                m = M_LIST[n]
                kt8, kt16 = kt_first if n == 0 else fetch(n)
                for bt in range(BT):
                    bsl = slice(bt * 128, (bt + 1) * 128)
                    acc = mps.tile([128, NFREE], f32, tag="acc")
                    for j in range(m // 2):
                        nc.tensor.matmul(
                            acc,
                            xT8[:, 2 * j : 2 * j + 2, bsl],
                            kt8[:, 2 * j : 2 * j + 2, :],
                            start=(j == 0),
                            stop=False,
                            perf_mode=DR,
                        )
                    for t in range(m, DT):
                        nc.tensor.matmul(
                            acc,
                            xT16[:, t - T16OFF, bsl],
                            kt16[:, t - m, :],
                            start=(m == 0 and t == 0),
                            stop=(t == DT - 1),
                        )
                    # epilogue: y[b, u] = sum_k acc[b, (u,k)] * c[b, k]
                    tmp = epp.tile([128, NFREE], f32, tag="tmp")
                    nc.vector.tensor_mul(tmp, acc, c_rep[:, bt, :])
                    yt = ypp.tile([128, UPT], f32, tag="yt")
                    nc.vector.tensor_reduce(
                        yt,
                        tmp.rearrange("p (u k) -> p u k", k=K),
                        axis=mybir.AxisListType.X,
                        op=mybir.AluOpType.add,
                    )
                    # y rings ride the gpsimd SWDGE queue: their wait on the
                    # epilogue semaphore must not gate the input streams, and
                    # the deep yt pool absorbs SWDGE latency
                    nc.gpsimd.dma_start(
                        y[bsl, n * UPT : (n + 1) * UPT],
                        yt,
                    )
    nc.compile()
    return nc


def _in_maps(x, c_prob, kernel, bias):
    np16 = mybir.dt.np(BF16)
    x = np.asarray(x, dtype=np.float32)
    c_prob = np.asarray(c_prob, dtype=np.float32)
    kernel = np.asarray(kernel, dtype=np.float32)

    xs = (x.T * np.float32(XSCALE))  # [D, B]
    x16_full = xs[T16OFF * 128 :].astype(np16)
    ks = kernel * np.float32(WSCALE)
    k16_full = ks[T16OFF * 128 :].astype(np16)
    cps = c_prob * np.float32(1.0 / (XSCALE * WSCALE))
    if MF8:
        np8 = mybir.dt.np(F8)
        x8_full = xs[:D8].astype(np8)
        k8_full = ks[:D8].astype(np8)  # [D8, U, K]

    maps = []
    for c in range(NCORES):
        bq, uh = c % SHARD_B, c // SHARD_B
        bsl = slice(bq * BS, (bq + 1) * BS)
        usl = slice(uh * USH, (uh + 1) * USH)
        mp = {
            "xt16": np.ascontiguousarray(x16_full[:, bsl]),
            "cp": np.ascontiguousarray(cps[bsl]),
            "kern16": np.ascontiguousarray(k16_full[:, usl, :]),
        }
        if MF8:
            mp["xt8"] = np.ascontiguousarray(x8_full[:, bsl])
            mp["kern8"] = np.ascontiguousarray(k8_full[:, usl, :])
        maps.append(mp)
    return maps


def _gather(results, bias):
    out = np.empty((B, U), dtype=np.float32)
    for c in range(NCORES):
        bq, uh = c % SHARD_B, c // SHARD_B
        out[bq * BS : (bq + 1) * BS, uh * USH : (uh + 1) * USH] = results[c]["y"]
    out += np.asarray(bias, dtype=np.float32)
    return out


def kernel(x, c_prob, kernel, bias):
    if "nc" not in _CACHE:
        _CACHE["nc"] = _build()
    nc = _CACHE["nc"]
    res = bass_utils.run_bass_kernel_spmd(
        nc, _in_maps(x, c_prob, kernel, bias), list(range(NCORES))
    )
    return _gather(res.results, bias)


# revision 44
# speedup vs baseline: 1.0069x; 1.0008x over previous
"""Trainium2 Bass kernel for nn_BasisDense: y = einsum('bd,duk,bk->bu', x, kernel, c_prob) + bias.

Strategy (v5):
  - Factorize: t[b,(u,k)] = x @ kernel2d (kernel2d = kernel.reshape(D, U*K)),
    then y[b,u] = sum_k t[b,u,k]*c_prob[b,k] (DVE epilogue); bias added on host.
  - All-bf16 matmuls (full PE rate, 216ns/512-row measured issue cadence; fp8
    DoubleRow measured at parity -- the moving stream is 1 elem/cycle either
    way -- so it is disabled by default via M_LIST). Operands pre-scaled by
    powers of two (x*16, kernel*256 -- exact in bf16) with the 1/4096 unscale
    folded into c_prob on the host, so an fp8 head can share the PSUM group.
  - Hybrid shard across 8 cores: batch B into 4 x units U into 2.
  - Queue discipline: each DMA-trigger ring costs ~600ns of engine time, so
    the kernel chunk stream is split across BOTH HWDGE queues (sync/scalar,
    alternating chunks) to keep ring pace ~300ns/chunk; x rides along in
    consumption order; y rings (which wait on the epilogue semaphore) go to
    the otherwise-idle gpsimd SWDGE queue with a deep yt pool to absorb its
    latency, so no input stream ever sits behind a semaphore wait.
  - ~13 warm-up matmuls on a gpsimd-memset tile run during the DMA head so
    the PE_HAM clock gate is released (2.4 GHz) before the real stream.
"""
import sys

sys.path.insert(0, "/opt/trn_rl_repo")

import numpy as np
import concourse.bacc as bacc
import concourse.mybir as mybir
import concourse.tile as tile
from concourse import bass_utils

B, D, U, K = 4096, 2048, 2048, 8
NCORES = 8
SHARD_U = 2  # units-dimension shards
SHARD_B = NCORES // SHARD_U
BS = B // SHARD_B  # batch rows per core
USH = U // SHARD_U  # units per core
UKS = USH * K  # fused (u,k) output columns per core
NFREE = 512  # matmul moving free dim (1 PSUM bank of fp32)
NT = UKS // NFREE  # n-tiles
DT = D // 128  # contraction d-tiles
BT = BS // 128  # batch partition-tiles per core
UPT = NFREE // K  # u-columns produced per n-tile
KT_BUFS = 5
WARM_MMS = 8  # PE warm-up matmuls issued during the DMA head

# fp8 d-tiles per n-tile (0, 2 or 4; DoubleRow pairs). A DoubleRow matmul
# measures ~110ns effective vs 432ns for the 2 bf16 matmuls it replaces
# (full 2x rate). rel err: all-2 -> 1.34e-2, [4,2] mix -> 1.63e-2,
# all-4 -> 1.88e-2 (gate 2e-2).
M_LIST = [2, 4] * (NT // 2)
MF8 = 4 if max(M_LIST) > 0 else 0  # d-tiles shipped in fp8
D8 = MF8 * 128  # fp8 d-rows
T16OFF = 2 if MF8 else 0  # first d-tile carried in bf16
DT16 = DT - T16OFF

XSCALE = 16.0  # x pre-scale (power of 2: exact in bf16)
WSCALE = 256.0  # kernel pre-scale
F8 = mybir.dt.float8e4
BF16 = mybir.dt.bfloat16

_CACHE = {}


def _build():
    nc = bacc.Bacc("TRN2", target_bir_lowering=False, debug=False, num_devices=NCORES)
    f32 = mybir.dt.float32
    DR = mybir.MatmulPerfMode.DoubleRow

    xt16 = nc.dram_tensor("xt16", [DT16 * 128, BS], BF16, kind="ExternalInput").ap()
    cp = nc.dram_tensor("cp", [BS, K], f32, kind="ExternalInput").ap()
    kern16 = nc.dram_tensor("kern16", [DT16 * 128, USH, K], BF16, kind="ExternalInput").ap()
    y = nc.dram_tensor("y", [BS, USH], f32, kind="ExternalOutput").ap()
    if MF8:
        xt8 = nc.dram_tensor("xt8", [D8, BS], F8, kind="ExternalInput").ap()
        kern8 = nc.dram_tensor("kern8", [D8, USH, K], F8, kind="ExternalInput").ap()
        x8v = xt8.rearrange("(t p) b -> p t b", p=128)  # [128, MF8, BS]
        k8v = kern8.rearrange("(t p) u k -> p t (u k)", p=128)  # [128, MF8, UKS]

    x16v = xt16.rearrange("(t p) b -> p t b", p=128)  # [128, DT16, BS]
    k16v = kern16.rearrange("(t p) u k -> p t (u k)", p=128)  # [128, DT16, UKS]

    with tile.TileContext(nc) as tc:
        with (
            tc.tile_pool(name="const", bufs=1) as constp,
            tc.tile_pool(name="kt8p", bufs=KT_BUFS) as ktp8,
            tc.tile_pool(name="kt16p", bufs=KT_BUFS) as ktp16,
            tc.tile_pool(name="mps", bufs=8, space="PSUM") as mps,
            tc.tile_pool(name="ep", bufs=6) as epp,
            tc.tile_pool(name="yp", bufs=32) as ypp,
        ):
            if MF8:
                xT8 = constp.tile([128, MF8, BS], F8)
            xT16 = constp.tile([128, DT16, BS], BF16)
            c_nat = constp.tile([128, BT, K], f32)
            c_rep = constp.tile([128, BT, NFREE], f32)
            warm = constp.tile([128, NFREE], BF16)

            # c_prob first on the sync queue (tiny, needed by bt0 epilogue)
            nc.sync.dma_start(c_nat, cp.rearrange("(bt p) k -> p bt k", p=128))

            # PE warm-up: releases the HAM clock gate during the DMA head.
            # The producer memset rides GpSimd (idle at startup). The warm-up
            # PSUM bank comes from the mps pool and is recycled by the real
            # accumulation groups.
            nc.gpsimd.memset(warm, 0.0)
            wpsum = mps.tile([128, NFREE], f32, tag="acc")
            for _ in range(WARM_MMS):
                nc.tensor.matmul(
                    wpsum, warm[:, 0:128], warm, start=True, stop=True,
                    skip_group_check=True,
                )

            # replicate c_prob 64x along the free dim on the DVE (tiny)
            for bt in range(BT):
                nc.vector.tensor_copy(c_rep[:, bt, 0:K], c_nat[:, bt, :])
                s = K
                while s < NFREE:
                    nc.vector.tensor_copy(c_rep[:, bt, s : 2 * s], c_rep[:, bt, 0:s])
                    s *= 2

            def fetch(n, first=False):
                m = M_LIST[n]
                kt8 = None
                if MF8:
                    kt8 = ktp8.tile([128, MF8, NFREE], F8, tag="kt8")
                    if m:
                        nc.scalar.dma_start(
                            kt8[:, 0:m, :],
                            k8v[:, 0:m, n * NFREE : (n + 1) * NFREE],
                        )
                kt16 = ktp16.tile([128, DT16, NFREE], BF16, tag="kt16")
                nsl = slice(n * NFREE, (n + 1) * NFREE)
                if first:
                    # Head schedule, hand-ordered for just-in-time supply of
                    # groups 0-1 while the clock is still cold. The first
                    # DoubleRow matmul is gated on xt8-bt0/kt8 (rung first);
                    # bt0's bf16 x arrives as two half-slices racing the
                    # per-tile kt chunk stream; bt1's slices ride ahead of
                    # the kt chunks so group 1 never waits.
                    half = DT16 // 2
                    if MF8:
                        nc.sync.dma_start(xT8[:, :, 0:128], x8v[:, :, 0:128])
                    nc.scalar.dma_start(xT16[:, 0:half, 0:128], x16v[:, 0:half, 0:128])
                    nc.sync.dma_start(
                        xT16[:, half:DT16, 0:128], x16v[:, half:DT16, 0:128]
                    )
                    for t in range(m, DT):
                        eng = nc.sync if t % 2 == 0 else nc.scalar
                        eng.dma_start(kt16[:, t - m, :], k16v[:, t - T16OFF, nsl])
                        if t == m + 3:
                            # bt1's x slices ride here: late enough not to
                            # starve group 0's chunk stream, early enough
                            # for group 1
                            nc.scalar.dma_start(
                                xT16[:, :, 128:256], x16v[:, :, 128:256]
                            )
                            if MF8:
                                nc.sync.dma_start(xT8[:, :, 128:256], x8v[:, :, 128:256])
                else:
                    # steady state: coarse 4-tile chunks, ~600ns of engine
                    # time per ring makes fewer+bigger rings strictly better
                    # (prefetch depth covers the arrival granularity)
                    ntiles = DT - m
                    q = 0
                    for lo in range(0, ntiles, 4):
                        hi = min(lo + 4, ntiles)
                        eng = nc.sync if q % 2 == 0 else nc.scalar
                        q += 1
                        eng.dma_start(
                            kt16[:, lo:hi, :], k16v[:, m - T16OFF + lo : m - T16OFF + hi, nsl]
                        )
                return kt8, kt16

            kt_first = fetch(0, first=True)

            # remaining x slices (bt 2..7), alternating queues
            for bt in range(2, BT):
                bsl = slice(bt * 128, (bt + 1) * 128)
                eng = nc.sync if bt % 2 == 0 else nc.scalar
                eng.dma_start(xT16[:, :, bsl], x16v[:, :, bsl])
                if MF8:
                    (nc.scalar if bt % 2 == 0 else nc.sync).dma_start(
                        xT8[:, :, bsl], x8v[:, :, bsl]
                    )

            for n in range(NT):
                m = M_LIST[n]
                kt8, kt16 = kt_first if n == 0 else fetch(n)
                for bt in range(BT):
                    bsl = slice(bt * 128, (bt + 1) * 128)
                    acc = mps.tile([128, NFREE], f32, tag="acc")
                    for j in range(m // 2):
                        nc.tensor.matmul(
                            acc,
                            xT8[:, 2 * j : 2 * j + 2, bsl],
                            kt8[:, 2 * j : 2 * j + 2, :],
                            start=(j == 0),
                            stop=False,
                            perf_mode=DR,
                        )
                    for t in range(m, DT):
                        nc.tensor.matmul(
                            acc,
                            xT16[:, t - T16OFF, bsl],
                            kt16[:, t - m, :],
                            start=(m == 0 and t == 0),
                            stop=(t == DT - 1),
                        )
                    # epilogue: y[b, u] = sum_k acc[b, (u,k)] * c[b, k]
                    tmp = epp.tile([128, NFREE], f32, tag="tmp")
                    nc.vector.tensor_mul(tmp, acc, c_rep[:, bt, :])
                    yt = ypp.tile([128, UPT], f32, tag="yt")
                    nc.vector.tensor_reduce(
                        yt,
                        tmp.rearrange("p (u k) -> p u k", k=K),
                        axis=mybir.AxisListType.X,
                        op=mybir.AluOpType.add,
                    )
                    # y rings ride the gpsimd SWDGE queue: their wait on the
                    # epilogue semaphore must not gate the input streams, and
                    # the deep yt pool absorbs SWDGE latency
                    nc.gpsimd.dma_start(
                        y[bsl, n * UPT : (n + 1) * UPT],
                        yt,
                    )
    nc.compile()
    return nc


def _in_maps(x, c_prob, kernel, bias):
    np16 = mybir.dt.np(BF16)
    x = np.asarray(x, dtype=np.float32)
    c_prob = np.asarray(c_prob, dtype=np.float32)
    kernel = np.asarray(kernel, dtype=np.float32)

    xs = (x.T * np.float32(XSCALE))  # [D, B]
    x16_full = xs[T16OFF * 128 :].astype(np16)
    ks = kernel * np.float32(WSCALE)
    k16_full = ks[T16OFF * 128 :].astype(np16)
    cps = c_prob * np.float32(1.0 / (XSCALE * WSCALE))
    if MF8:
        np8 = mybir.dt.np(F8)
        x8_full = xs[:D8].astype(np8)
        k8_full = ks[:D8].astype(np8)  # [D8, U, K]

    maps = []
    for c in range(NCORES):
        bq, uh = c % SHARD_B, c // SHARD_B
        bsl = slice(bq * BS, (bq + 1) * BS)
        usl = slice(uh * USH, (uh + 1) * USH)
        mp = {
            "xt16": np.ascontiguousarray(x16_full[:, bsl]),
            "cp": np.ascontiguousarray(cps[bsl]),
            "kern16": np.ascontiguousarray(k16_full[:, usl, :]),
        }
        if MF8:
            mp["xt8"] = np.ascontiguousarray(x8_full[:, bsl])
            mp["kern8"] = np.ascontiguousarray(k8_full[:, usl, :])
        maps.append(mp)
    return maps


def _gather(results, bias):
    out = np.empty((B, U), dtype=np.float32)
    for c in range(NCORES):
        bq, uh = c % SHARD_B, c // SHARD_B
        out[bq * BS : (bq + 1) * BS, uh * USH : (uh + 1) * USH] = results[c]["y"]
    out += np.asarray(bias, dtype=np.float32)
    return out


def kernel(x, c_prob, kernel, bias):
    if "nc" not in _CACHE:
        _CACHE["nc"] = _build()
    nc = _CACHE["nc"]
    res = bass_utils.run_bass_kernel_spmd(
        nc, _in_maps(x, c_prob, kernel, bias), list(range(NCORES))
    )
    return _gather(res.results, bias)


# revision 46
# speedup vs baseline: 1.2293x; 1.2208x over previous
"""Trainium2 Bass kernel for nn_BasisDense: y = einsum('bd,duk,bk->bu', x, kernel, c_prob) + bias.

Strategy (v5):
  - Factorize: t[b,(u,k)] = x @ kernel2d (kernel2d = kernel.reshape(D, U*K)),
    then y[b,u] = sum_k t[b,u,k]*c_prob[b,k] (DVE epilogue); bias added on host.
  - All-bf16 matmuls (full PE rate, 216ns/512-row measured issue cadence; fp8
    DoubleRow measured at parity -- the moving stream is 1 elem/cycle either
    way -- so it is disabled by default via M_LIST). Operands pre-scaled by
    powers of two (x*16, kernel*256 -- exact in bf16) with the 1/4096 unscale
    folded into c_prob on the host, so an fp8 head can share the PSUM group.
  - Hybrid shard across 8 cores: batch B into 4 x units U into 2.
  - Queue discipline: each DMA-trigger ring costs ~600ns of engine time, so
    the kernel chunk stream is split across BOTH HWDGE queues (sync/scalar,
    alternating chunks) to keep ring pace ~300ns/chunk; x rides along in
    consumption order; y rings (which wait on the epilogue semaphore) go to
    the otherwise-idle gpsimd SWDGE queue with a deep yt pool to absorb its
    latency, so no input stream ever sits behind a semaphore wait.
  - ~13 warm-up matmuls on a gpsimd-memset tile run during the DMA head so
    the PE_HAM clock gate is released (2.4 GHz) before the real stream.
"""
import sys

sys.path.insert(0, "/opt/trn_rl_repo")

import numpy as np
import concourse.bacc as bacc
import concourse.mybir as mybir
import concourse.tile as tile
from concourse import bass_utils

B, D, U, K = 4096, 2048, 2048, 8
NCORES = 8
SHARD_U = 2  # units-dimension shards
SHARD_B = NCORES // SHARD_U
BS = B // SHARD_B  # batch rows per core
USH = U // SHARD_U  # units per core
UKS = USH * K  # fused (u,k) output columns per core
NFREE = 512  # matmul moving free dim (1 PSUM bank of fp32)
NT = UKS // NFREE  # n-tiles
DT = D // 128  # contraction d-tiles
BT = BS // 128  # batch partition-tiles per core
UPT = NFREE // K  # u-columns produced per n-tile
KT_BUFS = 5
WARM_MMS = 8  # PE warm-up matmuls issued during the DMA head

# fp8 d-tiles per n-tile (0, 2 or 4; DoubleRow pairs). A DoubleRow matmul
# measures ~110ns effective vs 432ns for the 2 bf16 matmuls it replaces
# (full 2x rate). rel err: all-2 -> 1.34e-2, [4,2] mix -> 1.63e-2,
# all-4 -> 1.88e-2 (gate 2e-2).
M_LIST = [4, 2, 4, 4] * (NT // 4)
MF8 = 4 if max(M_LIST) > 0 else 0  # d-tiles shipped in fp8
D8 = MF8 * 128  # fp8 d-rows
T16OFF = 2 if MF8 else 0  # first d-tile carried in bf16
DT16 = DT - T16OFF

XSCALE = 16.0  # x pre-scale (power of 2: exact in bf16)
WSCALE = 256.0  # kernel pre-scale
F8 = mybir.dt.float8e4
BF16 = mybir.dt.bfloat16

_CACHE = {}


def _build():
    nc = bacc.Bacc("TRN2", target_bir_lowering=False, debug=False, num_devices=NCORES)
    f32 = mybir.dt.float32
    DR = mybir.MatmulPerfMode.DoubleRow

    xt16 = nc.dram_tensor("xt16", [DT16 * 128, BS], BF16, kind="ExternalInput").ap()
    cp = nc.dram_tensor("cp", [BS, K], f32, kind="ExternalInput").ap()
    kern16 = nc.dram_tensor("kern16", [DT16 * 128, USH, K], BF16, kind="ExternalInput").ap()
    y = nc.dram_tensor("y", [BS, USH], f32, kind="ExternalOutput").ap()
    if MF8:
        xt8 = nc.dram_tensor("xt8", [D8, BS], F8, kind="ExternalInput").ap()
        kern8 = nc.dram_tensor("kern8", [D8, USH, K], F8, kind="ExternalInput").ap()
        x8v = xt8.rearrange("(t p) b -> p t b", p=128)  # [128, MF8, BS]
        k8v = kern8.rearrange("(t p) u k -> p t (u k)", p=128)  # [128, MF8, UKS]

    x16v = xt16.rearrange("(t p) b -> p t b", p=128)  # [128, DT16, BS]
    k16v = kern16.rearrange("(t p) u k -> p t (u k)", p=128)  # [128, DT16, UKS]

    with tile.TileContext(nc) as tc:
        with (
            tc.tile_pool(name="const", bufs=1) as constp,
            tc.tile_pool(name="kt8p", bufs=KT_BUFS) as ktp8,
            tc.tile_pool(name="kt16p", bufs=KT_BUFS) as ktp16,
            tc.tile_pool(name="mps", bufs=8, space="PSUM") as mps,
            tc.tile_pool(name="ep", bufs=6) as epp,
            tc.tile_pool(name="yp", bufs=32) as ypp,
        ):
            if MF8:
                xT8 = constp.tile([128, MF8, BS], F8)
            xT16 = constp.tile([128, DT16, BS], BF16)
            c_nat = constp.tile([128, BT, K], f32)
            c_rep = constp.tile([128, BT, NFREE], f32)
            warm = constp.tile([128, NFREE], BF16)

            # c_prob first on the sync queue (tiny, needed by bt0 epilogue)
            nc.sync.dma_start(c_nat, cp.rearrange("(bt p) k -> p bt k", p=128))

            # PE warm-up: releases the HAM clock gate during the DMA head.
            # The producer memset rides GpSimd (idle at startup). The warm-up
            # PSUM bank comes from the mps pool and is recycled by the real
            # accumulation groups.
            nc.gpsimd.memset(warm, 0.0)
            wpsum = mps.tile([128, NFREE], f32, tag="acc")
            for _ in range(WARM_MMS):
                nc.tensor.matmul(
                    wpsum, warm[:, 0:128], warm, start=True, stop=True,
                    skip_group_check=True,
                )

            # replicate c_prob 64x along the free dim on the DVE (tiny)
            for bt in range(BT):
                nc.vector.tensor_copy(c_rep[:, bt, 0:K], c_nat[:, bt, :])
                s = K
                while s < NFREE:
                    nc.vector.tensor_copy(c_rep[:, bt, s : 2 * s], c_rep[:, bt, 0:s])
                    s *= 2

            def fetch(n, first=False):
                m = M_LIST[n]
                kt8 = None
                if MF8:
                    kt8 = ktp8.tile([128, MF8, NFREE], F8, tag="kt8")
                    if m:
                        nc.scalar.dma_start(
                            kt8[:, 0:m, :],
                            k8v[:, 0:m, n * NFREE : (n + 1) * NFREE],
                        )
                kt16 = ktp16.tile([128, DT16, NFREE], BF16, tag="kt16")
                nsl = slice(n * NFREE, (n + 1) * NFREE)
                if first:
                    # Head schedule, hand-ordered for just-in-time supply of
                    # groups 0-1 while the clock is still cold. The first
                    # DoubleRow matmul is gated on xt8-bt0/kt8 (rung first);
                    # bt0's bf16 x arrives as two half-slices racing the
                    # per-tile kt chunk stream; bt1's slices ride ahead of
                    # the kt chunks so group 1 never waits.
                    half = DT16 // 2
                    if MF8:
                        nc.sync.dma_start(xT8[:, :, 0:128], x8v[:, :, 0:128])
                    nc.scalar.dma_start(xT16[:, 0:half, 0:128], x16v[:, 0:half, 0:128])
                    nc.sync.dma_start(
                        xT16[:, half:DT16, 0:128], x16v[:, half:DT16, 0:128]
                    )
                    for t in range(m, DT):
                        eng = nc.sync if t % 2 == 0 else nc.scalar
                        eng.dma_start(kt16[:, t - m, :], k16v[:, t - T16OFF, nsl])
                        if t == m + 3:
                            # bt1's x slices ride here: late enough not to
                            # starve group 0's chunk stream, early enough
                            # for group 1
                            nc.scalar.dma_start(
                                xT16[:, :, 128:256], x16v[:, :, 128:256]
                            )
                            if MF8:
                                nc.sync.dma_start(xT8[:, :, 128:256], x8v[:, :, 128:256])
                else:
                    # steady state: coarse 4-tile chunks, ~600ns of engine
                    # time per ring makes fewer+bigger rings strictly better
                    # (prefetch depth covers the arrival granularity)
                    ntiles = DT - m
                    q = 0
                    for lo in range(0, ntiles, 4):
                        hi = min(lo + 4, ntiles)
                        eng = nc.sync if q % 2 == 0 else nc.scalar
                        q += 1
                        eng.dma_start(
                            kt16[:, lo:hi, :], k16v[:, m - T16OFF + lo : m - T16OFF + hi, nsl]
                        )
                return kt8, kt16

            kt_first = fetch(0, first=True)

            # remaining x slices (bt 2..7), alternating queues
            for bt in range(2, BT):
                bsl = slice(bt * 128, (bt + 1) * 128)
                eng = nc.sync if bt % 2 == 0 else nc.scalar
                eng.dma_start(xT16[:, :, bsl], x16v[:, :, bsl])
                if MF8:
                    (nc.scalar if bt % 2 == 0 else nc.sync).dma_start(
                        xT8[:, :, bsl], x8v[:, :, bsl]
                    )

            for n in range(NT):
                m = M_LIST[n]
                kt8, kt16 = kt_first if n == 0 else fetch(n)
                for bt in range(BT):
                    bsl = slice(bt * 128, (bt + 1) * 128)
                    acc = mps.tile([128, NFREE], f32, tag="acc")
                    for j in range(m // 2):
                        nc.tensor.matmul(
                            acc,
                            xT8[:, 2 * j : 2 * j + 2, bsl],
                            kt8[:, 2 * j : 2 * j + 2, :],
                            start=(j == 0),
                            stop=False,
                            perf_mode=DR,
                        )
                    for t in range(m, DT):
                        nc.tensor.matmul(
                            acc,
                            xT16[:, t - T16OFF, bsl],
                            kt16[:, t - m, :],
                            start=(m == 0 and t == 0),
                            stop=(t == DT - 1),
                        )
                    # epilogue: y[b, u] = sum_k acc[b, (u,k)] * c[b, k]
                    tmp = epp.tile([128, NFREE], f32, tag="tmp")
                    nc.vector.tensor_mul(tmp, acc, c_rep[:, bt, :])
                    yt = ypp.tile([128, UPT], f32, tag="yt")
                    nc.vector.tensor_reduce(
                        yt,
                        tmp.rearrange("p (u k) -> p u k", k=K),
                        axis=mybir.AxisListType.X,
                        op=mybir.AluOpType.add,
                    )
                    # y rings ride the gpsimd SWDGE queue: their wait on the
                    # epilogue semaphore must not gate the input streams, and
                    # the deep yt pool absorbs SWDGE latency. The last n-tile
                    # switches to the (by then idle) sync HWDGE queue, whose
                    # lower latency shortens the drain after the final group.
                    eng = nc.sync if n == NT - 1 else nc.gpsimd
                    eng.dma_start(
                        y[bsl, n * UPT : (n + 1) * UPT],
                        yt,
                    )
    nc.compile()
    return nc


def _in_maps(x, c_prob, kernel, bias):
    np16 = mybir.dt.np(BF16)
    x = np.asarray(x, dtype=np.float32)
    c_prob = np.asarray(c_prob, dtype=np.float32)
    kernel = np.asarray(kernel, dtype=np.float32)

    xs = (x.T * np.float32(XSCALE))  # [D, B]
    x16_full = xs[T16OFF * 128 :].astype(np16)
    ks = kernel * np.float32(WSCALE)
    k16_full = ks[T16OFF * 128 :].astype(np16)
    cps = c_prob * np.float32(1.0 / (XSCALE * WSCALE))
    if MF8:
        np8 = mybir.dt.np(F8)
        x8_full = xs[:D8].astype(np8)
        k8_full = ks[:D8].astype(np8)  # [D8, U, K]

    maps = []
    for c in range(NCORES):
        bq, uh = c % SHARD_B, c // SHARD_B
        bsl = slice(bq * BS, (bq + 1) * BS)
        usl = slice(uh * USH, (uh + 1) * USH)
        mp = {
            "xt16": np.ascontiguousarray(x16_full[:, bsl]),
            "cp": np.ascontiguousarray(cps[bsl]),
            "kern16": np.ascontiguousarray(k16_full[:, usl, :]),
        }
        if MF8:
            mp["xt8"] = np.ascontiguousarray(x8_full[:, bsl])
            mp["kern8"] = np.ascontiguousarray(k8_full[:, usl, :])
        maps.append(mp)
    return maps


def _gather(results, bias):
    out = np.empty((B, U), dtype=np.float32)
    for c in range(NCORES):
        bq, uh = c % SHARD_B, c // SHARD_B
        out[bq * BS : (bq + 1) * BS, uh * USH : (uh + 1) * USH] = results[c]["y"]
    out += np.asarray(bias, dtype=np.float32)
    return out


def kernel(x, c_prob, kernel, bias):
    if "nc" not in _CACHE:
        _CACHE["nc"] = _build()
    nc = _CACHE["nc"]
    res = bass_utils.run_bass_kernel_spmd(
        nc, _in_maps(x, c_prob, kernel, bias), list(range(NCORES))
    )
    return _gather(res.results, bias)
